# revision 9
# baseline (speedup 1.0000x reference)
"""Single-head masked attention (B=4, S=2048, D=1024, fp32) on 8 TRN2 NeuronCores.

Sharding: core c handles batch b=c//2, query half h=c%2 (1024 queries).
Each core computes K/V for all 2048 keys of its batch (duplicated across the
pair of cores sharing a batch; a pair-AllGather exchange was measured slower
-- the 2-core collective costs ~100us exposed on this runtime).
For h=1 cores the key axis is rotated by 1024 on the host so that every core
runs the identical SPMD program (attention output is invariant to key
permutation when the mask is permuted identically).

Host pre-transposes x and W so that all on-device matmuls contract along the
partition dimension with zero on-chip transposes:
  Q^T[e,q] = WqT.T @ xT[:, :1024]     (lhsT=WqT [d,e], rhs=xT [d,q])
  K^T[e,k] = WkT.T @ xT               (lhsT=WkT [d,e], rhs=xT [d,k])
  V[k,dv]  = xT.T  @ WvT              (lhsT=xT  [d,k], rhs=WvT [d,dv])
Scores are computed TRANSPOSED, [k_part, q_free]:
  S^T = K^T.T @ Q^T   (lhsT=K^T slice [e,k128], rhs=Q^T [e,q512])
so the masked softmax numerator is ONE fused ScalarE op per tile:
  attnU^T = exp(S^T * (1/sqrt(D)) + mask_bias[k])   (per-partition bias;
masked lanes get -30000 -> exp underflows to exactly 0 == the -inf mask
semantics; no max-subtraction needed since |scores/32| <~ 6, fp32-safe).
attnU^T [k,q] is directly the lhsT of the PV matmul (no transposes anywhere):
  out[q,dv] = attnU^T.T @ V
Row sums of exp come from PE matmuls against a ones vector (N=2: fp32r needs
an even moving free dim) and the final normalize + bv bias is one DVE op:
  out = psum * (1/sumexp)[q] + bv_bcast
(softmax weights sum to 1, so bv lands exactly once per row; this removes the
free-dim bias broadcast from the V projection).

All matmuls run in float32r (fp32 bits, replicated PE mode: 1 cycle/row at
free dim >= 256 vs 4 for plain fp32; ~1.6e-4 component error; HW-verified to
accept raw fp32 bit patterns from DRAM directly).

Queue discipline (v2 lesson: HWDGE issue is in-order per engine, so a compute
op waiting on a semaphore blocks every DMA issue queued behind it):
  sync   = W loads (split by e-column group so the first matmul group needs
           only 0.5 MB of W), V spill + V stream + output writes
  scalar = x chunk loads + small constants (ACT runs only the phase-2 exps)
  vector = all PSUM->SBUF movement (bias adds, V casts, final normalize)
V is spilled to DRAM after projection and streamed back during PV (SBUF
cannot hold x, W, Q^T, K^T, V and attnU^T simultaneously).
"""

from contextlib import ExitStack

import numpy as np

import concourse.bacc as bacc
import concourse.mybir as mybir
import concourse.tile as tile
from concourse.bass_utils import run_bass_kernel_spmd

D = 1024       # model dim = head dim
S = 2048       # sequence length (keys per core)
QL = 1024      # queries per core
N_CORES = 8
SCALE = 1.0 / 32.0   # 1/sqrt(D)
MASK_NEG = -30000.0

F32 = mybir.dt.float32
F32R = mybir.dt.float32r
AF = mybir.ActivationFunctionType
ALU = mybir.AluOpType


def _build_nc():
    nc = bacc.Bacc(None)

    xT = nc.declare_dram_parameter("xT", [D, S], F32R, isOutput=False)[:]
    wqT = nc.declare_dram_parameter("wqT", [D, D], F32R, isOutput=False)[:]
    wkT = nc.declare_dram_parameter("wkT", [D, D], F32R, isOutput=False)[:]
    wvT = nc.declare_dram_parameter("wvT", [D, D], F32R, isOutput=False)[:]
    bqT = nc.declare_dram_parameter("bqT", [128, 8], F32, isOutput=False)[:]
    bkT = nc.declare_dram_parameter("bkT", [128, 8], F32, isOutput=False)[:]
    mbT = nc.declare_dram_parameter("mbT", [128, 16], F32, isOutput=False)[:]
    bvb = nc.declare_dram_parameter("bvb", [128, D], F32, isOutput=False)[:]
    onesd = nc.declare_dram_parameter("onesd", [128, 2], F32R, isOutput=False)[:]
    out_d = nc.declare_dram_parameter("out", [QL, D], F32, isOutput=True)[:]

    vspill = nc.dram_tensor("vspill", [S, D], F32R)[:]

    with tile.TileContext(nc) as tc:
        _emit(nc, tc, xT, wqT, wkT, wvT, bqT, bkT, mbT, bvb, onesd,
              out_d, vspill)
    nc.finalize()
    return nc


def _emit(nc, tc, xT, wqT, wkT, wvT, bqT, bkT, mbT, bvb, onesd, out_d, vspill):
    with ExitStack() as ctx:
        consts = ctx.enter_context(tc.tile_pool(name="consts", bufs=1))

        # Q^T [e,q] and K^T [e,k], stored as 8 e-partition tiles each.
        qkpool = ctx.enter_context(tc.tile_pool(name="qk", bufs=8))
        qt = [qkpool.tile([128, QL], F32R, tag="qt", name=f"qt{m}")
              for m in range(8)]
        kt = [qkpool.tile([128, S], F32R, tag="kt", name=f"kt{m}")
              for m in range(8)]

        # ---------------- Phase 1: projections ----------------
        with (
            tc.tile_pool(name="proj", bufs=1) as pp,
            tc.tile_pool(name="projps", bufs=6, space="PSUM") as pps,
        ):
            dma_insts = {}

            def load_w(wT, nm):
                # One [128, 8dk, 128e] tile per e-column group: matmul group m
                # only waits on its own 0.5 MB slice.
                ws = []
                for m in range(8):
                    w = pp.tile([128, 8, 128], F32R, tag="w", bufs=16,
                                name=f"{nm}{m}")
                    di = nc.sync.dma_start(
                        out=w,
                        in_=wT[:, m * 128:(m + 1) * 128]
                        .rearrange("(a p) e -> p a e", p=128))
                    dma_insts[f"{nm}{m}"] = di
                    ws.append(w)
                return ws

            def load_x_chunk(c0, nm):
                x = pp.tile([128, 8, 512], F32R, tag="x", bufs=2, name=nm)
                di = nc.scalar.dma_start(
                    out=x,
                    in_=xT[:, c0:c0 + 512].rearrange("(a p) s -> p a s", p=128))
                dma_insts[nm] = di
                return x

            # ---- Q^T = WqT.T @ xT[:, 0:1024]  (+ bq per-partition) ----
            wq = load_w(wqT, "wq")
            xq = [load_x_chunk(0, "xq0")]
            bq_sb = consts.tile([128, 8], F32, tag="bq", name="bq_sb")
            nc.scalar.dma_start(out=bq_sb, in_=bqT)
            bk_sb = consts.tile([128, 8], F32, tag="bk", name="bk_sb")
            nc.scalar.dma_start(out=bk_sb, in_=bkT)
            mb_sb = consts.tile([128, 16], F32, tag="mb", name="mb_sb")
            nc.scalar.dma_start(out=mb_sb, in_=mbT)
            ones_sb = consts.tile([128, 2], F32R, tag="ones", name="ones_sb")
            nc.scalar.dma_start(out=ones_sb, in_=onesd)
            # Preload the exp table set while the PE is in the projections.
            warm = consts.tile([128, 2], F32, tag="warm", name="warm")
            nc.scalar.activation(warm, ones_sb, AF.Exp)

            for qc in range(2):
                if qc + 1 < 2:
                    xq.append(load_x_chunk((qc + 1) * 512, f"xq{qc + 1}"))
                for m in range(8):
                    ps = pps.tile([128, 512], F32, tag="ps", name=f"psq{qc}_{m}")
                    for dk in range(8):
                        mm = nc.tensor.matmul(
                            ps, wq[m][:, dk, :], xq[qc][:, dk, :],
                            start=(dk == 0), stop=(dk == 7))
                        if qc == 0 and m == 0 and dk == 0:
                            # Hold the very first matmul until the whole first
                            # working set is resident, so the PE starts once
                            # and streams densely (a stuttering DMA-paced
                            # start keeps the HAM clock gate cold).
                            for dep in ("wq7", "xq0", "xq1"):
                                tile.add_dep_helper(
                                    mm.ins, dma_insts[dep].ins,
                                    reason="dense-start prefetch")
                    nc.vector.tensor_scalar_add(
                        qt[m][:, qc * 512:(qc + 1) * 512], ps, bq_sb[:, m:m + 1])

            # ---- K^T = WkT.T @ xT  (+ bk per-partition) ----
            wk = load_w(wkT, "wk")
            xk = [load_x_chunk(0, "xk0")]
            for kc in range(4):
                if kc + 1 < 4:
                    xk.append(load_x_chunk((kc + 1) * 512, f"xk{kc + 1}"))
                for m in range(8):
                    ps = pps.tile([128, 512], F32, tag="ps", name=f"psk{kc}_{m}")
                    for dk in range(8):
                        nc.tensor.matmul(
                            ps, wk[m][:, dk, :], xk[kc][:, dk, :],
                            start=(dk == 0), stop=(dk == 7))
                    nc.vector.tensor_scalar_add(
                        kt[m][:, kc * 512:(kc + 1) * 512], ps, bk_sb[:, m:m + 1])

            # ---- V = xT.T @ WvT, spilled to DRAM (bv added at the end) ----
            # wv is split per d-chunk (not per e-column) because V's matmul
            # free dim runs along Wv^T's e axis in 512-wide chunks. The tiles
            # are 4 KB/partition either way, so they share the "w" tag and
            # recycle the wq slots that died after the Q projection.
            wv = []
            for dk in range(8):
                w = pp.tile([128, D], F32R, tag="w", bufs=16, name=f"wv{dk}")
                nc.sync.dma_start(out=w, in_=wvT[dk * 128:(dk + 1) * 128, :])
                wv.append(w)
            xv = [load_x_chunk(0, "xv0")]
            for kc in range(4):
                if kc + 1 < 4:
                    xv.append(load_x_chunk((kc + 1) * 512, f"xv{kc + 1}"))
                for t4 in range(4):
                    krow = kc * 512 + t4 * 128
                    vst = pp.tile([128, D], F32R, tag="vst", bufs=2,
                                  name=f"vst{kc}_{t4}")
                    for dvc in range(2):
                        ps = pps.tile([128, 512], F32, tag="ps",
                                      name=f"psv{kc}_{t4}_{dvc}")
                        for dk in range(8):
                            nc.tensor.matmul(
                                ps,
                                xv[kc][:, dk, t4 * 128:(t4 + 1) * 128],
                                wv[dk][:, dvc * 512:(dvc + 1) * 512],
                                start=(dk == 0), stop=(dk == 7))
                        nc.vector.tensor_copy(
                            vst[:, dvc * 512:(dvc + 1) * 512], ps)
                    nc.sync.dma_start(out=vspill[krow:krow + 128, :], in_=vst)

        # ---------------- Phase 2: attention ----------------
        with (
            tc.tile_pool(name="att", bufs=1) as at_p,
            tc.tile_pool(name="attps", bufs=2, space="PSUM") as aps,
        ):
            bvb_sb = at_p.tile([128, D], F32, tag="bvb", bufs=1, name="bvb_sb")
            nc.scalar.dma_start(out=bvb_sb, in_=bvb)
            for qc in range(2):
                qsl = slice(qc * 512, (qc + 1) * 512)
                # scores^T -> fused mask+exp, one [k128, q512] tile per k-tile
                at = []
                for kt_i in range(16):
                    ps = aps.tile([128, 512], F32, tag="ps_s", bufs=2,
                                  name=f"pss{qc}_{kt_i}")
                    for ec in range(8):
                        nc.tensor.matmul(
                            ps,
                            kt[ec][:, kt_i * 128:(kt_i + 1) * 128],
                            qt[ec][:, qsl],
                            start=(ec == 0), stop=(ec == 7))
                    a = at_p.tile([128, 512], F32R, tag="at", bufs=32,
                                  name=f"at{qc}_{kt_i}")
                    nc.scalar.activation(
                        a, ps, AF.Exp,
                        bias=mb_sb[:, kt_i:kt_i + 1], scale=SCALE)
                    at.append(a)
                # sumexp over k (partition dim) via ones-matmul, then 1/x
                recips = []
                for qs in range(4):
                    pss = aps.tile([128, 2], F32, tag="ps_sum", bufs=2,
                                   name=f"pssum{qc}_{qs}")
                    for kt_i in range(16):
                        nc.tensor.matmul(
                            pss,
                            at[kt_i][:, qs * 128:(qs + 1) * 128],
                            ones_sb,
                            start=(kt_i == 0), stop=(kt_i == 15))
                    r = at_p.tile([128, 1], F32, tag="recip", bufs=8,
                                  name=f"r{qc}_{qs}")
                    nc.vector.reciprocal(r, pss[:, 0:1])
                    recips.append(r)
                # out[q,dv] = (attnU^T.T @ V) * recip[q] + bv
                for dvc in range(2):
                    dsl = slice(dvc * 512, (dvc + 1) * 512)
                    pvs = [aps.tile([128, 512], F32, tag="ps_pv", bufs=4,
                                    name=f"pspv{qc}_{dvc}_{qs}")
                           for qs in range(4)]
                    for kt_i in range(16):
                        v = at_p.tile([128, 512], F32R, tag="v", bufs=6,
                                      name=f"v{qc}_{dvc}_{kt_i}")
                        nc.sync.dma_start(
                            out=v,
                            in_=vspill[kt_i * 128:(kt_i + 1) * 128, dsl])
                        for qs in range(4):
                            nc.tensor.matmul(
                                pvs[qs],
                                at[kt_i][:, qs * 128:(qs + 1) * 128],
                                v,
                                start=(kt_i == 0), stop=(kt_i == 15))
                    for qs in range(4):
                        o = at_p.tile([128, 512], F32, tag="o", bufs=4,
                                      name=f"o{qc}_{dvc}_{qs}")
                        nc.vector.scalar_tensor_tensor(
                            o, pvs[qs], recips[qs], bvb_sb[:, dsl],
                            op0=ALU.mult, op1=ALU.add)
                        row = (qc * 4 + qs) * 128
                        nc.sync.dma_start(
                            out=out_d[row:row + 128, dsl], in_=o)


def _prep_inputs(x, mask, Wq, bq, Wk, bk, Wv, bv):
    x = np.ascontiguousarray(np.asarray(x, dtype=np.float32))
    mask = np.asarray(mask, dtype=bool)
    Wq = np.asarray(Wq, dtype=np.float32)
    bq = np.asarray(bq, dtype=np.float32)
    Wk = np.asarray(Wk, dtype=np.float32)
    bk = np.asarray(bk, dtype=np.float32)
    Wv = np.asarray(Wv, dtype=np.float32)
    bv = np.asarray(bv, dtype=np.float32)

    wqT = np.ascontiguousarray(Wq.T)
    wkT = np.ascontiguousarray(Wk.T)
    wvT = np.ascontiguousarray(Wv.T)
    bqT = np.ascontiguousarray(bq.reshape(8, 128).T)
    bkT = np.ascontiguousarray(bk.reshape(8, 128).T)
    bvb = np.ascontiguousarray(np.broadcast_to(bv, (128, D)))
    ones = np.ones((128, 2), dtype=np.float32)

    in_maps = []
    for c in range(N_CORES):
        b, h = divmod(c, 2)
        xTb = x[b].T  # [D, S] view
        if h == 0:
            xT_c = np.ascontiguousarray(xTb)
            mask_c = mask[b]
        else:
            xT_c = np.ascontiguousarray(
                np.concatenate([xTb[:, QL:], xTb[:, :QL]], axis=1))
            mask_c = np.concatenate([mask[b, QL:], mask[b, :QL]])
        mb = np.where(mask_c, 0.0, MASK_NEG).astype(np.float32)
        mbT = np.ascontiguousarray(mb.reshape(16, 128).T)
        in_maps.append({
            "xT": xT_c, "wqT": wqT, "wkT": wkT, "wvT": wvT,
            "bqT": bqT, "bkT": bkT, "mbT": mbT, "bvb": bvb, "onesd": ones,
        })
    return in_maps


def run(x, mask, Wq, bq, Wk, bk, Wv, bv, trace=False):
    """Build + run; returns (output, BassKernelResults)."""
    in_maps = _prep_inputs(x, mask, Wq, bq, Wk, bk, Wv, bv)
    nc = _build_nc()
    res = run_bass_kernel_spmd(nc, in_maps, list(range(N_CORES)), trace=trace)
    out = np.empty((4, S, D), dtype=np.float32)
    for c in range(N_CORES):
        b, h = divmod(c, 2)
        out[b, h * QL:(h + 1) * QL, :] = res.results[c]["out"]
    return out, res


def kernel(x, mask, Wq, bq, Wk, bk, Wv, bv):
    out, _ = run(x, mask, Wq, bq, Wk, bk, Wv, bv)
    return out


# revision 10
# speedup vs baseline: 1.0300x; 1.0300x over previous
"""Single-head masked attention (B=4, S=2048, D=1024, fp32) on 8 TRN2 NeuronCores.

Sharding: core c handles batch b=c//2, query half h=c%2 (1024 queries).
Each core computes K/V for all 2048 keys of its batch (duplicated across the
pair of cores sharing a batch; a pair-AllGather exchange was measured slower
-- the 2-core collective costs ~100us exposed on this runtime).
For h=1 cores the key axis is rotated by 1024 on the host so that every core
runs the identical SPMD program (attention output is invariant to key
permutation when the mask is permuted identically).

Host pre-transposes x and W so that all on-device matmuls contract along the
partition dimension with zero on-chip transposes:
  Q^T[e,q] = WqT.T @ xT[:, :1024]     (lhsT=WqT [d,e], rhs=xT [d,q])
  K^T[e,k] = WkT.T @ xT               (lhsT=WkT [d,e], rhs=xT [d,k])
  V[k,dv]  = xT.T  @ WvT              (lhsT=xT  [d,k], rhs=WvT [d,dv])
Scores are computed TRANSPOSED, [k_part, q_free]:
  S^T = K^T.T @ Q^T   (lhsT=K^T slice [e,k128], rhs=Q^T [e,q512])
so the masked softmax numerator is ONE fused ScalarE op per tile:
  attnU^T = exp(S^T * (1/sqrt(D)) + mask_bias[k])   (per-partition bias;
masked lanes get -30000 -> exp underflows to exactly 0 == the -inf mask
semantics; no max-subtraction needed since |scores/32| <~ 6, fp32-safe).
attnU^T [k,q] is directly the lhsT of the PV matmul (no transposes anywhere):
  out[q,dv] = attnU^T.T @ V
Row sums of exp come from PE matmuls against a ones vector (N=2: fp32r needs
an even moving free dim) and the final normalize + bv bias is one DVE op:
  out = psum * (1/sumexp)[q] + bv_bcast
(softmax weights sum to 1, so bv lands exactly once per row; this removes the
free-dim bias broadcast from the V projection).

All matmuls run in float32r (fp32 bits, replicated PE mode: 1 cycle/row at
free dim >= 256 vs 4 for plain fp32; ~1.6e-4 component error; HW-verified to
accept raw fp32 bit patterns from DRAM directly).

Queue discipline (v2 lesson: HWDGE issue is in-order per engine, so a compute
op waiting on a semaphore blocks every DMA issue queued behind it):
  sync   = W loads (split by e-column group so the first matmul group needs
           only 0.5 MB of W), V spill + V stream + output writes
  scalar = x chunk loads + small constants (ACT runs only the phase-2 exps)
  vector = all PSUM->SBUF movement (bias adds, V casts, final normalize)
V is spilled to DRAM after projection and streamed back during PV (SBUF
cannot hold x, W, Q^T, K^T, V and attnU^T simultaneously).
"""

from contextlib import ExitStack

import numpy as np

import concourse.bacc as bacc
import concourse.mybir as mybir
import concourse.tile as tile
from concourse.bass_utils import run_bass_kernel_spmd

D = 1024       # model dim = head dim
S = 2048       # sequence length (keys per core)
QL = 1024      # queries per core
N_CORES = 8
SCALE = 1.0 / 32.0   # 1/sqrt(D)
MASK_NEG = -30000.0

F32 = mybir.dt.float32
F32R = mybir.dt.float32r
AF = mybir.ActivationFunctionType
ALU = mybir.AluOpType


def _build_nc():
    nc = bacc.Bacc(None)

    xT = nc.declare_dram_parameter("xT", [D, S], F32R, isOutput=False)[:]
    wqT = nc.declare_dram_parameter("wqT", [D, D], F32R, isOutput=False)[:]
    wkT = nc.declare_dram_parameter("wkT", [D, D], F32R, isOutput=False)[:]
    wvT = nc.declare_dram_parameter("wvT", [D, D], F32R, isOutput=False)[:]
    bqT = nc.declare_dram_parameter("bqT", [128, 8], F32, isOutput=False)[:]
    bkT = nc.declare_dram_parameter("bkT", [128, 8], F32, isOutput=False)[:]
    mbT = nc.declare_dram_parameter("mbT", [128, 16], F32, isOutput=False)[:]
    bvb = nc.declare_dram_parameter("bvb", [128, D], F32, isOutput=False)[:]
    onesd = nc.declare_dram_parameter("onesd", [128, 2], F32R, isOutput=False)[:]
    out_d = nc.declare_dram_parameter("out", [QL, D], F32, isOutput=True)[:]

    vspill = nc.dram_tensor("vspill", [S, D], F32R)[:]

    with tile.TileContext(nc) as tc:
        _emit(nc, tc, xT, wqT, wkT, wvT, bqT, bkT, mbT, bvb, onesd,
              out_d, vspill)
    nc.finalize()
    return nc


def _emit(nc, tc, xT, wqT, wkT, wvT, bqT, bkT, mbT, bvb, onesd, out_d, vspill):
    with ExitStack() as ctx:
        consts = ctx.enter_context(tc.tile_pool(name="consts", bufs=1))

        # Q^T [e,q] and K^T [e,k], stored as 8 e-partition tiles each.
        qkpool = ctx.enter_context(tc.tile_pool(name="qk", bufs=8))
        qt = [qkpool.tile([128, QL], F32R, tag="qt", name=f"qt{m}")
              for m in range(8)]
        kt = [qkpool.tile([128, S], F32R, tag="kt", name=f"kt{m}")
              for m in range(8)]

        # ---------------- Phase 1: projections ----------------
        with (
            tc.tile_pool(name="proj", bufs=1) as pp,
            tc.tile_pool(name="projps", bufs=6, space="PSUM") as pps,
        ):
            dma_insts = {}

            def load_w(wT, nm):
                # One [128, 8dk, 128e] tile per e-column group: matmul group m
                # only waits on its own 0.5 MB slice.
                ws = []
                for m in range(8):
                    w = pp.tile([128, 8, 128], F32R, tag="w", bufs=16,
                                name=f"{nm}{m}")
                    di = nc.sync.dma_start(
                        out=w,
                        in_=wT[:, m * 128:(m + 1) * 128]
                        .rearrange("(a p) e -> p a e", p=128))
                    dma_insts[f"{nm}{m}"] = di
                    ws.append(w)
                return ws

            def load_x_chunk(c0, nm):
                x = pp.tile([128, 8, 512], F32R, tag="x", bufs=2, name=nm)
                di = nc.scalar.dma_start(
                    out=x,
                    in_=xT[:, c0:c0 + 512].rearrange("(a p) s -> p a s", p=128))
                dma_insts[nm] = di
                return x

            # ---- Q^T = WqT.T @ xT[:, 0:1024]  (+ bq per-partition) ----
            wq = load_w(wqT, "wq")
            xq = [load_x_chunk(0, "xq0")]
            bq_sb = consts.tile([128, 8], F32, tag="bq", name="bq_sb")
            nc.scalar.dma_start(out=bq_sb, in_=bqT)
            bk_sb = consts.tile([128, 8], F32, tag="bk", name="bk_sb")
            nc.scalar.dma_start(out=bk_sb, in_=bkT)
            mb_sb = consts.tile([128, 16], F32, tag="mb", name="mb_sb")
            nc.scalar.dma_start(out=mb_sb, in_=mbT)
            ones_sb = consts.tile([128, 2], F32R, tag="ones", name="ones_sb")
            nc.scalar.dma_start(out=ones_sb, in_=onesd)
            # Preload the exp table set while the PE is in the projections.
            warm = consts.tile([128, 2], F32, tag="warm", name="warm")
            nc.scalar.activation(warm, ones_sb, AF.Exp)

            for qc in range(2):
                if qc + 1 < 2:
                    xq.append(load_x_chunk((qc + 1) * 512, f"xq{qc + 1}"))
                for m in range(8):
                    ps = pps.tile([128, 512], F32, tag="ps", name=f"psq{qc}_{m}")
                    for dk in range(8):
                        nc.tensor.matmul(
                            ps, wq[m][:, dk, :], xq[qc][:, dk, :],
                            start=(dk == 0), stop=(dk == 7))
                    nc.vector.tensor_scalar_add(
                        qt[m][:, qc * 512:(qc + 1) * 512], ps, bq_sb[:, m:m + 1])

            # ---- K^T = WkT.T @ xT  (+ bk per-partition) ----
            wk = load_w(wkT, "wk")
            xk = [load_x_chunk(0, "xk0")]
            for kc in range(4):
                if kc + 1 < 4:
                    xk.append(load_x_chunk((kc + 1) * 512, f"xk{kc + 1}"))
                for m in range(8):
                    ps = pps.tile([128, 512], F32, tag="ps", name=f"psk{kc}_{m}")
                    for dk in range(8):
                        nc.tensor.matmul(
                            ps, wk[m][:, dk, :], xk[kc][:, dk, :],
                            start=(dk == 0), stop=(dk == 7))
                    nc.vector.tensor_scalar_add(
                        kt[m][:, kc * 512:(kc + 1) * 512], ps, bk_sb[:, m:m + 1])

            # ---- V = xT.T @ WvT, spilled to DRAM (bv added at the end) ----
            # wv is split per d-chunk (not per e-column) because V's matmul
            # free dim runs along Wv^T's e axis in 512-wide chunks. The tiles
            # are 4 KB/partition either way, so they share the "w" tag and
            # recycle the wq slots that died after the Q projection.
            wv = []
            for dk in range(8):
                w = pp.tile([128, D], F32R, tag="w", bufs=16, name=f"wv{dk}")
                nc.sync.dma_start(out=w, in_=wvT[dk * 128:(dk + 1) * 128, :])
                wv.append(w)
            xv = [load_x_chunk(0, "xv0")]
            for kc in range(4):
                if kc + 1 < 4:
                    xv.append(load_x_chunk((kc + 1) * 512, f"xv{kc + 1}"))
                for t4 in range(4):
                    krow = kc * 512 + t4 * 128
                    vst = pp.tile([128, D], F32R, tag="vst", bufs=2,
                                  name=f"vst{kc}_{t4}")
                    for dvc in range(2):
                        ps = pps.tile([128, 512], F32, tag="ps",
                                      name=f"psv{kc}_{t4}_{dvc}")
                        for dk in range(8):
                            nc.tensor.matmul(
                                ps,
                                xv[kc][:, dk, t4 * 128:(t4 + 1) * 128],
                                wv[dk][:, dvc * 512:(dvc + 1) * 512],
                                start=(dk == 0), stop=(dk == 7))
                        nc.vector.tensor_copy(
                            vst[:, dvc * 512:(dvc + 1) * 512], ps)
                    nc.sync.dma_start(out=vspill[krow:krow + 128, :], in_=vst)

        # ---------------- Phase 2: attention ----------------
        with (
            tc.tile_pool(name="att", bufs=1) as at_p,
            tc.tile_pool(name="attps", bufs=2, space="PSUM") as aps,
        ):
            bvb_sb = at_p.tile([128, D], F32, tag="bvb", bufs=1, name="bvb_sb")
            nc.scalar.dma_start(out=bvb_sb, in_=bvb)
            for qc in range(2):
                qsl = slice(qc * 512, (qc + 1) * 512)
                # scores^T -> fused mask+exp, one [k128, q512] tile per k-tile
                at = []
                for kt_i in range(16):
                    ps = aps.tile([128, 512], F32, tag="ps_s", bufs=2,
                                  name=f"pss{qc}_{kt_i}")
                    for ec in range(8):
                        nc.tensor.matmul(
                            ps,
                            kt[ec][:, kt_i * 128:(kt_i + 1) * 128],
                            qt[ec][:, qsl],
                            start=(ec == 0), stop=(ec == 7))
                    a = at_p.tile([128, 512], F32R, tag="at", bufs=32,
                                  name=f"at{qc}_{kt_i}")
                    nc.scalar.activation(
                        a, ps, AF.Exp,
                        bias=mb_sb[:, kt_i:kt_i + 1], scale=SCALE)
                    at.append(a)
                # sumexp over k (partition dim) via ones-matmul, then 1/x
                recips = []
                for qs in range(4):
                    pss = aps.tile([128, 2], F32, tag="ps_sum", bufs=2,
                                   name=f"pssum{qc}_{qs}")
                    for kt_i in range(16):
                        nc.tensor.matmul(
                            pss,
                            at[kt_i][:, qs * 128:(qs + 1) * 128],
                            ones_sb,
                            start=(kt_i == 0), stop=(kt_i == 15))
                    r = at_p.tile([128, 1], F32, tag="recip", bufs=8,
                                  name=f"r{qc}_{qs}")
                    nc.vector.reciprocal(r, pss[:, 0:1])
                    recips.append(r)
                # out[q,dv] = (attnU^T.T @ V) * recip[q] + bv
                for dvc in range(2):
                    dsl = slice(dvc * 512, (dvc + 1) * 512)
                    pvs = [aps.tile([128, 512], F32, tag="ps_pv", bufs=4,
                                    name=f"pspv{qc}_{dvc}_{qs}")
                           for qs in range(4)]
                    for kt_i in range(16):
                        v = at_p.tile([128, 512], F32R, tag="v", bufs=6,
                                      name=f"v{qc}_{dvc}_{kt_i}")
                        nc.sync.dma_start(
                            out=v,
                            in_=vspill[kt_i * 128:(kt_i + 1) * 128, dsl])
                        for qs in range(4):
                            nc.tensor.matmul(
                                pvs[qs],
                                at[kt_i][:, qs * 128:(qs + 1) * 128],
                                v,
                                start=(kt_i == 0), stop=(kt_i == 15))
                    for qs in range(4):
                        o = at_p.tile([128, 512], F32, tag="o", bufs=4,
                                      name=f"o{qc}_{dvc}_{qs}")
                        nc.vector.scalar_tensor_tensor(
                            o, pvs[qs], recips[qs], bvb_sb[:, dsl],
                            op0=ALU.mult, op1=ALU.add)
                        row = (qc * 4 + qs) * 128
                        nc.sync.dma_start(
                            out=out_d[row:row + 128, dsl], in_=o)


def _prep_inputs(x, mask, Wq, bq, Wk, bk, Wv, bv):
    x = np.ascontiguousarray(np.asarray(x, dtype=np.float32))
    mask = np.asarray(mask, dtype=bool)
    Wq = np.asarray(Wq, dtype=np.float32)
    bq = np.asarray(bq, dtype=np.float32)
    Wk = np.asarray(Wk, dtype=np.float32)
    bk = np.asarray(bk, dtype=np.float32)
    Wv = np.asarray(Wv, dtype=np.float32)
    bv = np.asarray(bv, dtype=np.float32)

    wqT = np.ascontiguousarray(Wq.T)
    wkT = np.ascontiguousarray(Wk.T)
    wvT = np.ascontiguousarray(Wv.T)
    bqT = np.ascontiguousarray(bq.reshape(8, 128).T)
    bkT = np.ascontiguousarray(bk.reshape(8, 128).T)
    bvb = np.ascontiguousarray(np.broadcast_to(bv, (128, D)))
    ones = np.ones((128, 2), dtype=np.float32)

    in_maps = []
    for c in range(N_CORES):
        b, h = divmod(c, 2)
        xTb = x[b].T  # [D, S] view
        if h == 0:
            xT_c = np.ascontiguousarray(xTb)
            mask_c = mask[b]
        else:
            xT_c = np.ascontiguousarray(
                np.concatenate([xTb[:, QL:], xTb[:, :QL]], axis=1))
            mask_c = np.concatenate([mask[b, QL:], mask[b, :QL]])
        mb = np.where(mask_c, 0.0, MASK_NEG).astype(np.float32)
        mbT = np.ascontiguousarray(mb.reshape(16, 128).T)
        in_maps.append({
            "xT": xT_c, "wqT": wqT, "wkT": wkT, "wvT": wvT,
            "bqT": bqT, "bkT": bkT, "mbT": mbT, "bvb": bvb, "onesd": ones,
        })
    return in_maps


def run(x, mask, Wq, bq, Wk, bk, Wv, bv, trace=False):
    """Build + run; returns (output, BassKernelResults)."""
    in_maps = _prep_inputs(x, mask, Wq, bq, Wk, bk, Wv, bv)
    nc = _build_nc()
    res = run_bass_kernel_spmd(nc, in_maps, list(range(N_CORES)), trace=trace)
    out = np.empty((4, S, D), dtype=np.float32)
    for c in range(N_CORES):
        b, h = divmod(c, 2)
        out[b, h * QL:(h + 1) * QL, :] = res.results[c]["out"]
    return out, res


def kernel(x, mask, Wq, bq, Wk, bk, Wv, bv):
    out, _ = run(x, mask, Wq, bq, Wk, bk, Wv, bv)
    return out


# revision 13
# speedup vs baseline: 1.2184x; 1.1829x over previous
"""Single-head masked attention (B=4, S=2048, D=1024, fp32) on 8 TRN2 NeuronCores.

Sharding: core c handles batch b=c//2, query half h=c%2 (1024 queries), with
K/V work over all 2048 keys of its batch. For h=1 cores the key axis is
rotated by 1024 on the host so every core runs the identical SPMD program
(attention is invariant to key permutation when the mask is permuted too).

The kernel exploits two algebraic reassociations that cut the matmul work
from 1280 to 1024 tile-matmuls per core:

1) scores^T = K @ Q^T = (x @ Wk^T + bk) @ Q^T
            = x @ (Wk^T @ Q^T)  [+ bk . Q^T, constant per query]
   The bias term is constant across keys for each query, so softmax's shift
   invariance cancels it EXACTLY -- bk is simply dropped. Computing
   G[d,q] = Wk^T @ Q^T first (2.1 GF) and then S^T = x @ G (4.3 GF) replaces
   K-projection (4.3) + scores (4.3). Bonus: G's lhsT is Wk in its NATIVE
   [e,d] layout, and K^T (8MB) is never materialized.

2) out = attnU @ (x @ Wv^T) / sumexp + bv
       = (attnU @ x) @ Wv^T / sumexp + bv
   Z^T[d,q] = x^T-weighted attention (4.3 GF) then out = Z^T.T @ Wv^T
   (2.1 GF) replaces V-projection (4.3) + PV (4.3). The value bias bv
   contributes exactly bv per row (softmax weights sum to 1) and is added in
   the final normalize op. V is never materialized (no DRAM spill).

Matmul layouts (contraction always on the partition dim, zero on-chip
transposes; host supplies xT=[d,s], xN=[s,d], wqT/wvT transposed, wkN native):
  Q^T[e,q]  : lhsT=WqT [d,e-col-tiles], rhs=xT [d,q]      (+bq per-partition)
  G[d,q]    : lhsT=WkN [e,d-slices],    rhs=Q^T [e,q]
  S^T[k,q]  : lhsT=xT  [d,k-slices],    rhs=G   [d,q]
  attnU^T   = exp(S^T/32 + mask_bias[k])  -- ONE fused ScalarE op per tile
              (masked lanes get -30000 -> exp underflows to exact 0; no
              max-subtraction needed: |s/32| <~ 6)
  sumexp[q] : lhsT=attnU^T [k,q-slices], rhs=ones [k,2]   (fp32r needs even N)
  Z^T[d,q]  : lhsT=xN [k,d-slices],      rhs=attnU^T [k,q]
  out[q,dv] : lhsT=Z^T [d,q-slices],     rhs=WvT [d,dv]
  final     : out = psum * (1/sumexp)[q] + bv_bcast  -- one DVE op

All matmuls run in float32r (fp32 bits at bf16-rate: 1 cycle/row for moving
free dim >= 256 vs 4 cycles/row for plain fp32; ~1.6e-4 component error;
HW-verified to accept raw fp32 bit patterns from DRAM directly).

Queue discipline (HWDGE issue is in-order per engine; a compute op waiting on
a semaphore would block DMA issues queued behind it): sync carries W loads +
xN streams + outputs; scalar carries x^T loads + constants (its only compute
is the phase-2 exps); vector does all PSUM->SBUF movement.
"""

from contextlib import ExitStack

import numpy as np

import concourse.bacc as bacc
import concourse.mybir as mybir
import concourse.tile as tile
from concourse.bass_utils import run_bass_kernel_spmd

D = 1024       # model dim = head dim
S = 2048       # sequence length (keys per core)
QL = 1024      # queries per core
N_CORES = 8
SCALE = 1.0 / 32.0   # 1/sqrt(D)
MASK_NEG = -30000.0

F32 = mybir.dt.float32
F32R = mybir.dt.float32r
AF = mybir.ActivationFunctionType
ALU = mybir.AluOpType


def _build_nc():
    nc = bacc.Bacc(None)

    xT = nc.declare_dram_parameter("xT", [D, S], F32R, isOutput=False)[:]
    xN = nc.declare_dram_parameter("xN", [S, D], F32R, isOutput=False)[:]
    wqT = nc.declare_dram_parameter("wqT", [D, D], F32R, isOutput=False)[:]
    wkN = nc.declare_dram_parameter("wkN", [D, D], F32R, isOutput=False)[:]
    wvT = nc.declare_dram_parameter("wvT", [D, D], F32R, isOutput=False)[:]
    bqT = nc.declare_dram_parameter("bqT", [128, 8], F32, isOutput=False)[:]
    mbT = nc.declare_dram_parameter("mbT", [128, 16], F32, isOutput=False)[:]
    bvb = nc.declare_dram_parameter("bvb", [128, D], F32, isOutput=False)[:]
    onesd = nc.declare_dram_parameter("onesd", [128, 2], F32R, isOutput=False)[:]
    out_d = nc.declare_dram_parameter("out", [QL, D], F32, isOutput=True)[:]

    with tile.TileContext(nc) as tc:
        _emit(nc, tc, xT, xN, wqT, wkN, wvT, bqT, mbT, bvb, onesd, out_d)
    nc.finalize()
    return nc


def _emit(nc, tc, xT, xN, wqT, wkN, wvT, bqT, mbT, bvb, onesd, out_d):
    with ExitStack() as ctx:
        consts = ctx.enter_context(tc.tile_pool(name="consts", bufs=1))

        # G[d,q] = Wk^T @ Q^T lives across both phases, 8 d-partition tiles.
        gpool = ctx.enter_context(tc.tile_pool(name="g", bufs=8))
        gt = [gpool.tile([128, QL], F32R, tag="gt", name=f"gt{m}")
              for m in range(8)]

        # ---------------- Phase 1: Q^T then G ----------------
        with (
            tc.tile_pool(name="proj", bufs=1) as pp,
            tc.tile_pool(name="projps", bufs=6, space="PSUM") as pps,
        ):
            # Q^T [e,q] as 8 e-partition tiles (phase-1 only).
            qt = [pp.tile([128, QL], F32R, tag="qt", bufs=8, name=f"qt{m}")
                  for m in range(8)]

            # wq split by e-column group so the first matmul group only waits
            # on its own 0.5 MB slice.
            wq = []
            for m in range(8):
                w = pp.tile([128, 8, 128], F32R, tag="w", bufs=16,
                            name=f"wq{m}")
                nc.sync.dma_start(
                    out=w,
                    in_=wqT[:, m * 128:(m + 1) * 128]
                    .rearrange("(a p) e -> p a e", p=128))
                wq.append(w)
            xq = []
            for c in range(2):
                x = pp.tile([128, 8, 512], F32R, tag="x", bufs=2, name=f"xq{c}")
                nc.scalar.dma_start(
                    out=x,
                    in_=xT[:, c * 512:(c + 1) * 512]
                    .rearrange("(a p) s -> p a s", p=128))
                xq.append(x)
            bq_sb = consts.tile([128, 8], F32, tag="bq", name="bq_sb")
            nc.scalar.dma_start(out=bq_sb, in_=bqT)
            mb_sb = consts.tile([128, 16], F32, tag="mb", name="mb_sb")
            nc.scalar.dma_start(out=mb_sb, in_=mbT)
            ones_sb = consts.tile([128, 2], F32R, tag="ones", name="ones_sb")
            nc.scalar.dma_start(out=ones_sb, in_=onesd)
            # Preload the exp table set while the PE is in the projections.
            warm = consts.tile([128, 2], F32, tag="warm", name="warm")
            nc.scalar.activation(warm, ones_sb, AF.Exp)

            # ---- Q^T = WqT.T @ xT[:, 0:1024]  (+ bq per-partition) ----
            for qc in range(2):
                for m in range(8):
                    ps = pps.tile([128, 512], F32, tag="ps", name=f"psq{qc}_{m}")
                    for dk in range(8):
                        nc.tensor.matmul(
                            ps, wq[m][:, dk, :], xq[qc][:, dk, :],
                            start=(dk == 0), stop=(dk == 7))
                    nc.vector.tensor_scalar_add(
                        qt[m][:, qc * 512:(qc + 1) * 512], ps, bq_sb[:, m:m + 1])

            # ---- G[d,q] = WkN.T @ Q^T  (Wk in native [e,d] layout) ----
            # wk tiles are [128e, 1024d] native rows: 4 KB/partition, same
            # slot size as the wq tiles, so they recycle the "w" tag slots.
            wk = []
            for ec in range(8):
                w = pp.tile([128, D], F32R, tag="w", bufs=16, name=f"wk{ec}")
                nc.sync.dma_start(out=w, in_=wkN[ec * 128:(ec + 1) * 128, :])
                wk.append(w)
            for dt in range(8):
                for qch in range(2):
                    ps = pps.tile([128, 512], F32, tag="ps",
                                  name=f"psg{dt}_{qch}")
                    for ec in range(8):
                        nc.tensor.matmul(
                            ps,
                            wk[ec][:, dt * 128:(dt + 1) * 128],
                            qt[ec][:, qch * 512:(qch + 1) * 512],
                            start=(ec == 0), stop=(ec == 7))
                    nc.vector.tensor_copy(
                        gt[dt][:, qch * 512:(qch + 1) * 512], ps)

        # ---------------- Phase 2: attention ----------------
        with (
            tc.tile_pool(name="att", bufs=1) as at_p,
            tc.tile_pool(name="aps1", bufs=2, space="PSUM") as aps,
            tc.tile_pool(name="aps2", bufs=4, space="PSUM") as zps,
        ):
            bvb_sb = at_p.tile([128, D], F32, tag="bvb", bufs=1, name="bvb_sb")
            nc.scalar.dma_start(out=bvb_sb, in_=bvb)
            # wv (= Wv^T rows, d-split) resident for the final out-matmul.
            wv = []
            for dt in range(8):
                w = at_p.tile([128, D], F32R, tag="wv", bufs=8, name=f"wv{dt}")
                nc.sync.dma_start(out=w, in_=wvT[dt * 128:(dt + 1) * 128, :])
                wv.append(w)

            # ---- S^T[k,q] = xT.T @ G -> fused mask+exp, both q-chunks ----
            at = [[], []]
            for kt_i in range(16):
                xs = at_p.tile([128, 8, 128], F32R, tag="xs", bufs=3,
                               name=f"xs{kt_i}")
                nc.scalar.dma_start(
                    out=xs,
                    in_=xT[:, kt_i * 128:(kt_i + 1) * 128]
                    .rearrange("(a p) s -> p a s", p=128))
                for qc in range(2):
                    ps = aps.tile([128, 512], F32, tag="ps_s", bufs=2,
                                  name=f"pss{qc}_{kt_i}")
                    for dc in range(8):
                        nc.tensor.matmul(
                            ps,
                            xs[:, dc, :],
                            gt[dc][:, qc * 512:(qc + 1) * 512],
                            start=(dc == 0), stop=(dc == 7))
                    a = at_p.tile([128, 512], F32R, tag="at", bufs=32,
                                  name=f"at{qc}_{kt_i}")
                    nc.scalar.activation(
                        a, ps, AF.Exp,
                        bias=mb_sb[:, kt_i:kt_i + 1], scale=SCALE)
                    at[qc].append(a)

            for qc in range(2):
                # sumexp over k (partition dim) via ones-matmul, then 1/x
                recips = []
                for qs in range(4):
                    pss = aps.tile([128, 2], F32, tag="ps_sum", bufs=2,
                                   name=f"pssum{qc}_{qs}")
                    for kt_i in range(16):
                        nc.tensor.matmul(
                            pss,
                            at[qc][kt_i][:, qs * 128:(qs + 1) * 128],
                            ones_sb,
                            start=(kt_i == 0), stop=(kt_i == 15))
                    r = at_p.tile([128, 1], F32, tag="recip", bufs=8,
                                  name=f"r{qc}_{qs}")
                    nc.vector.reciprocal(r, pss[:, 0:1])
                    recips.append(r)

                # ---- Z^T[d,q] = xN.T @ attnU^T (4 d-tiles per xN pass) ----
                zt = []
                for dth in range(2):
                    pzs = [zps.tile([128, 512], F32, tag="ps_z",
                                    name=f"psz{qc}_{dth}_{j}")
                           for j in range(4)]
                    for kt_i in range(16):
                        xn = at_p.tile([128, 512], F32R, tag="xn", bufs=4,
                                       name=f"xn{qc}_{dth}_{kt_i}")
                        nc.sync.dma_start(
                            out=xn,
                            in_=xN[kt_i * 128:(kt_i + 1) * 128,
                                   dth * 512:(dth + 1) * 512])
                        for j in range(4):
                            nc.tensor.matmul(
                                pzs[j],
                                xn[:, j * 128:(j + 1) * 128],
                                at[qc][kt_i],
                                start=(kt_i == 0), stop=(kt_i == 15))
                    for j in range(4):
                        z = at_p.tile([128, 512], F32R, tag="zt", bufs=8,
                                      name=f"zt{qc}_{dth}_{j}")
                        nc.vector.tensor_copy(z, pzs[j])
                        zt.append(z)

                # ---- out[q,dv] = Z^T.T @ WvT * recip[q] + bv ----
                for qs in range(4):
                    for dvc in range(2):
                        ps = zps.tile([128, 512], F32, tag="ps_z",
                                      name=f"pso{qc}_{qs}_{dvc}")
                        for dt in range(8):
                            nc.tensor.matmul(
                                ps,
                                zt[dt][:, qs * 128:(qs + 1) * 128],
                                wv[dt][:, dvc * 512:(dvc + 1) * 512],
                                start=(dt == 0), stop=(dt == 7))
                        o = at_p.tile([128, 512], F32, tag="o", bufs=4,
                                      name=f"o{qc}_{qs}_{dvc}")
                        nc.vector.scalar_tensor_tensor(
                            o, ps, recips[qs], bvb_sb[:, dvc * 512:(dvc + 1) * 512],
                            op0=ALU.mult, op1=ALU.add)
                        row = (qc * 4 + qs) * 128
                        nc.sync.dma_start(
                            out=out_d[row:row + 128, dvc * 512:(dvc + 1) * 512],
                            in_=o)


def _prep_inputs(x, mask, Wq, bq, Wk, bk, Wv, bv):
    x = np.ascontiguousarray(np.asarray(x, dtype=np.float32))
    mask = np.asarray(mask, dtype=bool)
    Wq = np.asarray(Wq, dtype=np.float32)
    bq = np.asarray(bq, dtype=np.float32)
    Wk = np.ascontiguousarray(np.asarray(Wk, dtype=np.float32))
    Wv = np.asarray(Wv, dtype=np.float32)
    bv = np.asarray(bv, dtype=np.float32)
    del bk  # exactly cancelled by softmax shift invariance

    wqT = np.ascontiguousarray(Wq.T)
    wvT = np.ascontiguousarray(Wv.T)
    bqT = np.ascontiguousarray(bq.reshape(8, 128).T)
    bvb = np.ascontiguousarray(np.broadcast_to(bv, (128, D)))
    ones = np.ones((128, 2), dtype=np.float32)

    in_maps = []
    for c in range(N_CORES):
        b, h = divmod(c, 2)
        if h == 0:
            xN_c = x[b]
            mask_c = mask[b]
        else:
            xN_c = np.concatenate([x[b, QL:], x[b, :QL]], axis=0)
            mask_c = np.concatenate([mask[b, QL:], mask[b, :QL]])
        xN_c = np.ascontiguousarray(xN_c)
        xT_c = np.ascontiguousarray(xN_c.T)
        mb = np.where(mask_c, 0.0, MASK_NEG).astype(np.float32)
        mbT = np.ascontiguousarray(mb.reshape(16, 128).T)
        in_maps.append({
            "xT": xT_c, "xN": xN_c, "wqT": wqT, "wkN": Wk, "wvT": wvT,
            "bqT": bqT, "mbT": mbT, "bvb": bvb, "onesd": ones,
        })
    return in_maps


def run(x, mask, Wq, bq, Wk, bk, Wv, bv, trace=False):
    """Build + run; returns (output, BassKernelResults)."""
    in_maps = _prep_inputs(x, mask, Wq, bq, Wk, bk, Wv, bv)
    nc = _build_nc()
    res = run_bass_kernel_spmd(nc, in_maps, list(range(N_CORES)), trace=trace)
    out = np.empty((4, S, D), dtype=np.float32)
    for c in range(N_CORES):
        b, h = divmod(c, 2)
        out[b, h * QL:(h + 1) * QL, :] = res.results[c]["out"]
    return out, res


def kernel(x, mask, Wq, bq, Wk, bk, Wv, bv):
    out, _ = run(x, mask, Wq, bq, Wk, bk, Wv, bv)
    return out
